# revision 3
# baseline (speedup 1.0000x reference)
"""MoE GroupedExperts kernel for 8 TRN2 NeuronCores.

Expert-parallel: expert e's tokens + weights go to core e. Tokens are
pre-sorted by expert, so routing is host-side slicing. Each core runs a
SwiGLU MLP: o = (silu(x @ gate) * (x @ up)) @ down.

Key optimizations over the naive version:
- All tensors are packed on the host into the exact SBUF layout
  (partition-major), so every DMA descriptor is a maximal contiguous
  run (4KB+ per partition) and the rings hit full bandwidth.
- Weight traffic is split across both HWDGE rings (sync + scalar) so
  the aggregate DMA rate exceeds a single ring's limit.
- A short burst of dummy matmuls at t=0 warms the PE HAM clock gate
  (idle default is 1.2 GHz; ~3.4us of activity unlocks 2.4 GHz), so
  the real GEMM stream runs at full clock from its first instruction.
- Per-token-tile output buffers are double-buffered so the PSUM->SBUF
  cast and the output DMA of tile 0 overlap the matmuls of tile 1.
"""

import sys

if "/opt/trn_rl_repo" not in sys.path:
    sys.path.insert(0, "/opt/trn_rl_repo")

import numpy as np

F16 = np.float16
E = 8
DIM = 1024
HID = 2048
N_CORES = 8
CPAD = 256          # tokens per expert per block (T/E for the target shape)
KC = DIM // 128     # 8 k-chunks for gate/up contraction
KH = HID // 128     # 16 k-chunks for down contraction
NCH = HID // 256    # 8 hid chunks (256 cols each) for gate/up streaming
CH = 256            # hid cols per chunk
PAIR = 2            # hid slices per PSUM bank (2*256 = 512 fp32)

_cache = {}


def _build():
    from concourse import bacc
    import concourse.tile as tile
    import concourse.mybir as mybir

    f32 = mybir.dt.float32
    f16 = mybir.dt.float16

    nc = bacc.Bacc("TRN2", target_bir_lowering=False, debug=False)
    # Packed DRAM layouts: partition dim first, contiguous per partition.
    xt_d = nc.dram_tensor("xt", [128, KC * CPAD], f16, kind="ExternalInput")
    gw_d = nc.dram_tensor("gw", [128, NCH * KC * CH], f16, kind="ExternalInput")
    uw_d = nc.dram_tensor("uw", [128, NCH * KC * CH], f16, kind="ExternalInput")
    dw_d = nc.dram_tensor("dw", [128, KH * DIM], f16, kind="ExternalInput")
    o_d = nc.dram_tensor("o", [CPAD, DIM], f16, kind="ExternalOutput")

    NTOK = CPAD // 128  # 2 token tiles
    NDC = DIM // 512    # 2 output column slices

    with tile.TileContext(nc) as tc:
        with (
            tc.tile_pool(name="sb", bufs=1) as sb,
            tc.tile_pool(name="stmp", bufs=2) as stmp_pool,
            tc.tile_pool(name="ht", bufs=NCH) as ht_pool,
            tc.tile_pool(name="outp", bufs=2) as out_pool,
            tc.tile_pool(name="psA", bufs=2, space="PSUM") as psA,
            tc.tile_pool(name="psB", bufs=2, space="PSUM") as psB,
            tc.tile_pool(name="psO", bufs=4, space="PSUM") as psO,
        ):
            xt_s = sb.tile([128, KC, CPAD], f16)
            gw_s = sb.tile([128, NCH, KC * CH], f16)
            uw_s = sb.tile([128, NCH, KC * CH], f16)
            dw_s = sb.tile([128, KH, DIM], f16)
            warm = sb.tile([128, 512], f16)

            # --- HAM warmup: dummy matmuls so the PE clock is at 2.4 GHz
            # by the time real data lands. ~8 N=512 MMs ~= 3.4us cold.
            nc.gpsimd.memset(warm[:], 0)
            wps = psO.tile([128, 512], f32, tag="po", name="warmps")
            for i in range(8):
                nc.tensor.matmul(
                    wps[:], warm[:, 0:128], warm[:],
                    start=True, stop=True, skip_group_check=True,
                )

            # --- DMA triggers, in consumption order, split across rings.
            # scalar ring: x first (needed by every MM), then up chunks.
            # sync ring: gate chunks. down halves go to both rings last.
            xt_v = xt_d.ap()
            gw_v = gw_d.ap()
            uw_v = uw_d.ap()
            dw_v = dw_d.ap()
            nc.scalar.dma_start(xt_s[:], xt_v.rearrange("p (k c) -> p k c", k=KC))
            for g in range(NCH):
                c0, c1 = g * KC * CH, (g + 1) * KC * CH
                nc.sync.dma_start(gw_s[:, g, :], gw_v[:, c0:c1])
                nc.scalar.dma_start(uw_s[:, g, :], uw_v[:, c0:c1])
            # down: k 0..7 on sync (2 chunks), k 8..15 on scalar (2 chunks)
            DK = KH // 4  # 4 k-chunks per DMA
            for h in range(2):
                k0, k1 = h * DK, (h + 1) * DK
                nc.sync.dma_start(
                    dw_s[:, k0:k1, :], dw_v[:, k0 * DIM:k1 * DIM]
                )
            for h in range(2, 4):
                k0, k1 = h * DK, (h + 1) * DK
                nc.scalar.dma_start(
                    dw_s[:, k0:k1, :], dw_v[:, k0 * DIM:k1 * DIM]
                )

            # --- gate/up grouped GEMMs; h produced in [hid, tok] layout.
            ht = []
            for g in range(NCH):
                pg = psA.tile([128, PAIR, CPAD], f32, tag="pg")
                pu = psB.tile([128, PAIR, CPAD], f32, tag="pu")
                for j in range(PAIR):
                    cj = j * 128
                    for k in range(KC):
                        nc.tensor.matmul(
                            pg[:, j, :], gw_s[:, g, k * CH + cj:k * CH + cj + 128],
                            xt_s[:, k, :],
                            start=(k == 0), stop=(k == KC - 1),
                            skip_group_check=True,
                        )
                for j in range(PAIR):
                    cj = j * 128
                    for k in range(KC):
                        nc.tensor.matmul(
                            pu[:, j, :], uw_s[:, g, k * CH + cj:k * CH + cj + 128],
                            xt_s[:, k, :],
                            start=(k == 0), stop=(k == KC - 1),
                            skip_group_check=True,
                        )
                stmp = stmp_pool.tile([128, PAIR, CPAD], f32, tag="stmp")
                nc.scalar.activation(
                    stmp[:], pg[:], mybir.ActivationFunctionType.Silu
                )
                ht_t = ht_pool.tile([128, PAIR, CPAD], f16, tag="ht")
                nc.vector.tensor_mul(ht_t[:], stmp[:], pu[:])
                ht.append(ht_t)

            # --- Down projection: o[tok, dim] = h @ down.
            for tok in range(NTOK):
                t0, t1 = tok * 128, (tok + 1) * 128
                po = [
                    psO.tile([128, 512], f32, tag="po", name=f"po{tok}_{dc}")
                    for dc in range(NDC)
                ]
                for k in range(KH):
                    for dc in range(NDC):
                        nc.tensor.matmul(
                            po[dc][:],
                            ht[k // PAIR][:, k % PAIR, t0:t1],
                            dw_s[:, k, dc * 512:(dc + 1) * 512],
                            start=(k == 0), stop=(k == KH - 1),
                            skip_group_check=True,
                        )
                out_s = out_pool.tile([128, DIM], f16, tag="out")
                for dc in range(NDC):
                    nc.vector.tensor_copy(
                        out_s[:, dc * 512:(dc + 1) * 512], po[dc][:]
                    )
                nc.scalar.dma_start(o_d[t0:t1, :], out_s[:])

    nc.compile()
    return nc


def _get_nc():
    if "nc" not in _cache:
        _cache["nc"] = _build()
    return _cache["nc"]


def _pack_x(xe):
    # xe [CPAD, DIM] fp16 -> [128, KC*CPAD]: [p][k][c], dim = k*128+p
    return np.ascontiguousarray(
        xe.T.reshape(KC, 128, CPAD).transpose(1, 0, 2).reshape(128, KC * CPAD)
    )


def _pack_gu(w):
    # w [DIM, HID] fp16 -> [128, NCH*KC*CH]: [p][g][k][c]
    return np.ascontiguousarray(
        w.reshape(KC, 128, NCH, CH).transpose(1, 2, 0, 3).reshape(128, -1)
    )


def _pack_dw(w):
    # w [HID, DIM] fp16 -> [128, KH*DIM]: [p][k][d]
    return np.ascontiguousarray(
        w.reshape(KH, 128, DIM).transpose(1, 0, 2).reshape(128, -1)
    )


def _run_block(nc, in_maps, collect):
    from concourse.bass_utils import run_bass_kernel_spmd

    kwargs = {} if collect is None else dict(collect.get("run_kwargs") or {})
    res = run_bass_kernel_spmd(nc, in_maps, core_ids=list(range(N_CORES)), **kwargs)
    if collect is not None:
        collect.setdefault("results", []).append(res)
    return [res.results[e]["o"] for e in range(E)]


def kernel(x, counts, gate_proj, up_proj, down_proj, _collect=None):
    x = np.asarray(x, dtype=np.float32).astype(F16)
    counts = np.asarray(counts, dtype=np.int32)
    gate_proj = np.asarray(gate_proj, dtype=np.float32).astype(F16)
    up_proj = np.asarray(up_proj, dtype=np.float32).astype(F16)
    down_proj = np.asarray(down_proj, dtype=np.float32).astype(F16)

    T = x.shape[0]
    offs = np.concatenate([[0], np.cumsum(counts)]).astype(np.int64)
    cmax = int(counts.max()) if counts.size else CPAD
    n_blocks = max(1, -(-cmax // CPAD))

    nc = _get_nc()
    wpacks = [
        {
            "gw": _pack_gu(gate_proj[e]),
            "uw": _pack_gu(up_proj[e]),
            "dw": _pack_dw(down_proj[e]),
        }
        for e in range(E)
    ]

    out = np.empty((T, DIM), dtype=np.float32)
    for b in range(n_blocks):
        in_maps = []
        spans = []
        for e in range(E):
            c = int(counts[e])
            s0 = min(b * CPAD, c)
            s1 = min((b + 1) * CPAD, c)
            xe = x[offs[e] + s0:offs[e] + s1]
            if xe.shape[0] < CPAD:
                xe = np.concatenate(
                    [xe, np.zeros((CPAD - xe.shape[0], DIM), F16)], axis=0
                )
            in_maps.append({"xt": _pack_x(xe), **wpacks[e]})
            spans.append((s0, s1))
        outs = _run_block(nc, in_maps, _collect)
        for e in range(E):
            s0, s1 = spans[e]
            if s1 > s0:
                out[offs[e] + s0:offs[e] + s1] = outs[e][: s1 - s0]
    return out
